# revision 9
# baseline (speedup 1.0000x reference)
"""Multi-head self-attention (B=4, S=2048, D=2048, H=16, hd=128) on 8 trn2
NeuronCores — flat-pipeline version.

Sharding: tensor-parallel over heads. Core c owns heads {2c, 2c+1}:
  - q/k projections for its 2 heads ([j, t] layout),
  - v projection computed directly in [t, j] layout (no PE transposes),
  - causal attention for its (4 batches x 2 heads) units,
  - partial output projection with its 256 rows of Wo (bf16 partials).
Host sums the 8 partial outputs and adds bo.

Design: one dense PE instruction stream. Attention units are emitted
c-major ((c,h0),(c,h1),...) and every spare PE slot is filled from a
token-bucket-paced FIFO of "fill thunks" (next batch's projection chains,
this batch's output-projection groups), so the PE never idles long enough
for HAM clock throttling.

Engine balance (PE is the bottleneck at ~91% busy, everything else is
arranged around keeping it fed):
  - softmax denominator: DVE pair-sum tree accumulated into one [128,512]
    bf16 tile, then gpsimd partition_all_reduce + one wide DVE
    reciprocal_approx_fast — zero PE instructions.
  - causal mask on diagonal tiles: per-tile gpsimd affine_select right
    after each exp (no DVE mask multiply, no mask constants).
  - v bias: pre-broadcast [128,512] constant fused into the PSUM->SBUF
    copy as a DVE tensor_add (no PE bias matmuls).
  - oproj PSUM->SBUF copies alternate DVE/ACT by group parity so neither
    engine's FIFO backs up; PSUM 'pp' pool has 4 banks (the old G bank).
  - wqkv is host-packed [p, group, k, 128] so each weight group is one
    contiguous-run DMA; the first group is split by k so the first
    projection chain starts ~8us after launch instead of ~19us.
Scores and AV matmuls on diagonal tiles are width-truncated to the causal
region.
"""

import math
from collections import deque

import numpy as np
import ml_dtypes

import concourse.bass as bass
import concourse.bacc as bacc
import concourse.mybir as mybir
import concourse.tile as tile
from concourse.bass_isa import ReduceOp
from concourse.bass_utils import run_bass_kernel_spmd

BF16 = mybir.dt.bfloat16
F32 = mybir.dt.float32

B, S, D_MODEL = 4, 2048, 2048
N_HEADS, HEAD_DIM = 16, 128
N_CORES = 8
H_PER = N_HEADS // N_CORES          # 2 heads per core
JL = H_PER * HEAD_DIM               # 256 local j-columns of q/k/v
T = B * S                           # 8192 tokens
KD = D_MODEL // 128                 # 16 contraction tiles over d_model
TC = S // 512                       # 4 token chunks of 512 per batch
SCALE = 1.0 / math.sqrt(HEAD_DIM)

_CACHED_NC = None


class _Fill:
    """FIFO of fill thunks with token-bucket-paced popping: a thunk is only
    emitted once enough budget has accumulated to cover its cost, so a big
    thunk is never injected into a short gap."""

    def __init__(self):
        self.q = deque()
        self.balance = 0.0

    def push(self, cost, key, thunk):
        self.q.append((cost, key, thunk))

    def pop_budget(self, budget):
        self.balance = min(self.balance + budget, 7.0)
        while self.q and self.q[0][0] <= self.balance:
            cost, _, thunk = self.q.popleft()
            thunk()
            self.balance -= cost

    def drain_key(self, pred):
        """Emit (in FIFO order) everything up to and including the last
        entry whose key matches pred."""
        if not any(pred(k) for _, k, _ in self.q):
            return
        last = max(i for i, (_, k, _) in enumerate(self.q) if pred(k))
        for _ in range(last + 1):
            _, _, thunk = self.q.popleft()
            thunk()

    def drain_all(self):
        while self.q:
            _, _, thunk = self.q.popleft()
            thunk()


def build_program():
    nc = bacc.Bacc("TRN2", target_bir_lowering=False, debug=False)

    xT = nc.dram_tensor("xT", [D_MODEL, T], BF16, kind="ExternalInput").ap()
    # host-packed: [p, group, k, 128] with groups (q.h0, q.h1, k.h0, k.h1,
    # v.h0, v.h1) — each group slice is one contiguous run per partition.
    wqkv = nc.dram_tensor("wqkv", [128, 6, KD, 128], BF16,
                          kind="ExternalInput").ap()
    bqkv = nc.dram_tensor("bqkv", [3 * JL], F32, kind="ExternalInput").ap()
    wo = nc.dram_tensor("wo", [JL, D_MODEL], BF16, kind="ExternalInput").ap()
    outT = nc.dram_tensor("outT", [D_MODEL, T], BF16, kind="ExternalOutput").ap()

    xT_r = xT.rearrange("(k p) t -> p k t", p=128)        # [128, KD, T]

    with tile.TileContext(nc) as tc:
        with (
            tc.tile_pool(name="const", bufs=1) as const,
            tc.tile_pool(name="work", bufs=1) as work,
            tc.tile_pool(name="psum", bufs=1, space="PSUM") as psum,
        ):
            # ---- constants ----
            wqkv_sb = const.tile([128, 6, KD, 128], BF16)
            bqkv_sb = const.tile([128, 4], F32)       # q/k biases, per-partition
            bv2row = const.tile([1, 512], F32)        # [bv, bv] doubled
            bv2row16 = const.tile([1, 512], BF16)
            bv2 = const.tile([128, 512], BF16)        # bv broadcast, fused add
            wo_sb = const.tile([128, JL // 128, D_MODEL], BF16)
            ones_col = const.tile([128, 1], BF16)     # G-reduce lhsT
            nc.gpsimd.memset(ones_col[:], 1.0)

            def load_trailing_consts():
                nc.sync.dma_start(wo_sb[:], wo.rearrange("(k p) d -> p k d", p=128))

            st = _State(nc, tc, work, psum, xT_r, wqkv_sb, bqkv_sb, bv2,
                        wo_sb, ones_col, outT, load_trailing_consts)

            st.push_proj(0)
            st.load_trailing_consts = None
            # Startup loads are spread over three DMA queues so they run
            # concurrently: x chunks on the SP queue, wqkv groups on the ACT
            # queue, biases on the DVE queue.  The jm0 weight group is split
            # by k so the first qk chain's first k-tiles land early.
            st._get_xt(0, 0)          # x chunk 0 heads the SP DMA queue
            for kc in range(4):
                nc.scalar.dma_start(wqkv_sb[:, 0, 4 * kc:4 * (kc + 1), :],
                                    wqkv[:, 0, 4 * kc:4 * (kc + 1), :])
            for g in range(1, 4):
                nc.scalar.dma_start(wqkv_sb[:, g], wqkv[:, g])
            nc.scalar.dma_start(wqkv_sb[:, 4:6], wqkv[:, 4:6])
            nc.sync.dma_start(
                bqkv_sb[:], bqkv[0:512].rearrange("(m p) -> p m", p=128))
            nc.sync.dma_start(
                bv2row[:, 0:256], bqkv[512:768].rearrange("(o j) -> o j", o=1))
            nc.sync.dma_start(
                bv2row[:, 256:512], bqkv[512:768].rearrange("(o j) -> o j", o=1))
            nc.vector.tensor_copy(bv2row16[:], bv2row[:])
            nc.gpsimd.partition_broadcast(bv2[:], bv2row16[:])
            st.load_trailing_consts = load_trailing_consts   # wo: next xt
            for b in range(B):
                if b + 1 < B:
                    st.push_proj(b + 1)
                st.emit_att_batch(b)
            st.finish()

    nc.compile()
    return nc


class _State:
    def __init__(self, nc, tc, work, psum, xT_r, wqkv_sb, bqkv_sb, bv2,
                 wo_sb, ones_col, outT, load_trailing_consts):
        self.nc = nc
        self.tc = tc
        self.work = work
        self.psum = psum
        self.xT_r = xT_r
        self.wqkv_sb = wqkv_sb
        self.bqkv_sb = bqkv_sb
        self.bv2 = bv2
        self.wo_sb = wo_sb
        self.ones_col = ones_col
        self.outT = outT
        self.load_trailing_consts = load_trailing_consts
        self.fill = _Fill()
        self.qkT = {}       # b -> tile [128, 4, S]  (q.h0, q.h1, k.h0, k.h1)
        self.v_sb = {}      # b -> tile [128, S//128, JL]
        self.yn = {}        # b -> tile [128, H_PER, S]
        self.xt = {}        # (b, tcn) -> tile [128, KD, 512]
        self.pending_norms = deque()
        self.oproj_ready = []

    # ---------------- projections (fill thunks) ----------------

    def _get_xt(self, b, tcn):
        key = (b, tcn)
        t = self.xt.get(key)
        if t is None:
            t = self.work.tile([128, KD, 512], BF16, tag="xt", bufs=2)
            ts = slice(b * S + tcn * 512, b * S + (tcn + 1) * 512)
            if b == 0 and tcn == 0:
                # split the very first load so chain mm k=0 starts after a
                # quarter of the transfer, not all of it
                for kc in range(4):
                    self.nc.sync.dma_start(
                        t[:, 4 * kc:4 * (kc + 1), :],
                        self.xT_r[:, 4 * kc:4 * (kc + 1), ts])
            else:
                self.nc.sync.dma_start(t[:], self.xT_r[:, :, ts])
            self.xt[key] = t
            if self.load_trailing_consts is not None:
                self.load_trailing_consts()
                self.load_trailing_consts = None
        return t

    def push_proj(self, b):
        nc = self.nc
        qkT = self.work.tile([128, 4, S], BF16, tag="qkT", bufs=2, name=f"qkT{b}")
        v_sb = self.work.tile([128, S // 128, JL], BF16, tag="v", bufs=2,
                              name=f"v{b}")
        self.qkT[b] = qkT
        self.v_sb[b] = v_sb
        self.yn[b] = self.work.tile([128, H_PER, S], BF16, tag="yn", bufs=2,
                                    name=f"yn{b}")

        def qk_chain(tcn, jm):
            def thunk():
              with nc.named_scope(f"proj.b{b}.t{tcn}"):
                xt = self._get_xt(b, tcn)
                if jm == 0 and tcn + 1 < TC:
                    self._get_xt(b, tcn + 1)          # prefetch next chunk
                ps = self.psum.tile([128, 512], F32, tag="pp", bufs=4)
                for k in range(KD):
                    nc.tensor.matmul(
                        ps[:],
                        lhsT=self.wqkv_sb[:, jm, k, :],
                        rhs=xt[:, k, :],
                        start=(k == 0), stop=(k == KD - 1),
                    )
                nc.vector.tensor_scalar_add(
                    qkT[:, jm, tcn * 512:(tcn + 1) * 512], ps[:],
                    self.bqkv_sb[:, jm:jm + 1],
                )
            return thunk

        def v_chain(m2):
            # two 128-token tiles (m = 2*m2, 2*m2+1) share one PSUM bank
            def thunk():
              with nc.named_scope(f"vproj.b{b}"):
                tcn = m2 // 2
                xt = self._get_xt(b, tcn)
                ps = self.psum.tile([128, 512], F32, tag="pp", bufs=4)
                for half in range(2):
                    off = ((2 * m2 + half) % 4) * 128
                    sl = slice(half * JL, (half + 1) * JL)
                    for k in range(KD):
                        nc.tensor.matmul(
                            ps[:, sl],
                            lhsT=xt[:, k, off:off + 128],
                            rhs=self.wqkv_sb[:, 4:6, k, :],
                            start=(k == 0), stop=(k == KD - 1),
                        )
                # bias fused into the PSUM->SBUF copy (bv2 = [bv, bv] bcast)
                nc.vector.tensor_add(
                    v_sb[:, 2 * m2:2 * m2 + 2, :], ps[:], self.bv2[:])
            return thunk

        for tcn in range(TC):
            for jm in range(4):
                self.fill.push(3.4, ('proj', b, tcn), qk_chain(tcn, jm))
            for i in range(2):
                self.fill.push(4.2, ('proj', b, tcn), v_chain(2 * tcn + i))

    def push_oproj(self, b, tcn):
        nc = self.nc
        yn = self.yn[b]
        t0 = b * S

        def group(dm):
            def thunk():
              with nc.named_scope(f"oproj.b{b}"):
                ps = self.psum.tile([128, 512], F32, tag="pp", bufs=4)
                for kj in range(JL // 128):
                    nc.tensor.matmul(
                        ps[:],
                        lhsT=self.wo_sb[:, kj, dm * 128:(dm + 1) * 128],
                        rhs=yn[:, kj, tcn * 512:(tcn + 1) * 512],
                        start=(kj == 0), stop=(kj == JL // 128 - 1),
                    )
                o_sb = self.work.tile([128, 512], BF16, tag="osb", bufs=10)
                # alternate engines so neither FIFO backs up behind copies
                if dm % 2 == 0:
                    nc.vector.tensor_copy(o_sb[:], ps[:])
                else:
                    nc.scalar.copy(o_sb[:], ps[:])
                # output stores get their own DMA queue (ACT dge) so they
                # never sit behind a 2MB xt input load on the SP queue
                nc.scalar.dma_start(
                    self.outT[dm * 128:(dm + 1) * 128,
                              t0 + tcn * 512: t0 + (tcn + 1) * 512],
                    o_sb[:],
                )
            return thunk

        for dm in range(D_MODEL // 128):
            self.fill.push(0.9, ('oproj', b, tcn), group(dm))

    # ---------------- attention ----------------

    def emit_norm_pending(self, min_backlog=0):
        # release oproj groups whose norm was emitted a full unit ago, so
        # their matmuls never wait on a norm-multiply still in the DVE queue
        for bc in self.oproj_ready:
            self.push_oproj(*bc)
        self.oproj_ready = []
        if len(self.pending_norms) <= min_backlog:
            return
        nc = self.nc
        b, c, h, u, rr = self.pending_norms.popleft()
        with nc.named_scope(f"norm.b{b}"):
            nc.vector.tensor_mul(
                self.yn[b][:, h, c * 512:(c + 1) * 512], u[:], rr[:])
        if h == H_PER - 1:
            self.oproj_ready.append((b, c))

    def emit_att_batch(self, b):
        nc = self.nc
        # last batch runs chunks big-to-small so the drain tail after the
        # final unit is the small c=0 unit's oproj, not the full-width c=3
        order = range(TC) if b + 1 < B else reversed(range(TC))
        for c in order:
            # everything this chunk's units read must already be emitted
            self.fill.drain_key(
                lambda k, b=b, c=c: k[0] == 'proj' and k[1] == b and k[2] <= c)
            for h in range(H_PER):
                self._emit_unit(b, c, h)
        # retire stragglers from older batches so tile bufs recycle
        self.fill.drain_key(lambda k, b=b: k[0] == 'oproj' and k[1] < b)

    def _emit_unit(self, b, c, h):
        nc = self.nc
        qkT = self.qkT[b]
        v_sb = self.v_sb[b]
        qT = qkT[:, h, c * 512:(c + 1) * 512]
        nm = 4 * (c + 1)
        nq = nm // 4
        u = self.psum.tile([128, 512], F32, tag="u", bufs=2)
        gacc = self.work.tile([128, 512], BF16, tag="gacc", bufs=2)

        boost = 1.6 if b == B - 1 else 1.0
        self.fill.pop_budget(1.5 * boost)
        with nc.named_scope(f"att.b{b}.c{c}.h{h}"):
            for qd in range(nq):
                e4 = self.work.tile([128, 4, 512], BF16, tag="e", bufs=6)
                diag = qd == nq - 1
                if diag:
                    for i in range(1, 4):   # zero the causally-skipped region
                        nc.gpsimd.memset(e4[:, i, 0:128 * i], 0.0)
                for i in range(4):
                    m = 4 * qd + i
                    off = 128 * i if diag else 0   # causal truncation
                    s2 = self.psum.tile([128, 512], F32, tag="s2", bufs=2)
                    nc.tensor.matmul(
                        s2[:, off:],
                        lhsT=qkT[:, 2 + h, m * 128:(m + 1) * 128],
                        rhs=qT[:, off:],
                        start=True, stop=True,
                    )
                    nc.scalar.activation(
                        e4[:, i, off:], s2[:, off:],
                        mybir.ActivationFunctionType.Exp, scale=SCALE)
                    if diag:
                        # causal triangle of this tile: zero cols < row idx
                        nc.gpsimd.affine_select(
                            out=e4[:, i, off:], in_=e4[:, i, off:],
                            compare_op=mybir.AluOpType.is_ge, fill=0.0,
                            base=0, pattern=[[1, 512 - off]],
                            channel_multiplier=-1,
                        )
                        self.fill.pop_budget(0.9 * boost)
                    elif i == 1:
                        self.fill.pop_budget(1.3 * boost)
                ps2 = self.work.tile([128, 2, 512], BF16, tag="ps2", bufs=2)
                nc.vector.tensor_add(ps2[:], e4[:, 0:2, :], e4[:, 2:4, :])
                # softmax denominator: accumulate pair-sums on DVE (no PE)
                if qd == 0:
                    nc.vector.tensor_add(gacc[:], ps2[:, 0, :], ps2[:, 1, :])
                else:
                    nc.vector.tensor_add(gacc[:], gacc[:], ps2[:, 0, :])
                    nc.vector.tensor_add(gacc[:], gacc[:], ps2[:, 1, :])
                for i in range(4):
                    m = 4 * qd + i
                    off = 128 * i if diag else 0
                    nc.tensor.matmul(
                        u[:, off:],
                        lhsT=v_sb[:, m, h * 128:(h + 1) * 128],
                        rhs=e4[:, i, off:],
                        start=(m == 0), stop=(m == nm - 1),
                        skip_group_check=True,
                    )
                self.fill.pop_budget(2.2 * boost)
                if qd == 0:
                    # norm of the previous unit, deferred past this unit's
                    # first quad so its DVE ops never head-block the queue
                    self.emit_norm_pending()
            # G = column-sum of gacc: one tiny PE matmul into an s2-pool
            # slot (fast latency; the dedicated G bank went to the pp pool)
            g = self.psum.tile([128, 512], F32, tag="s2", bufs=2)
            nc.tensor.matmul(g[0:1, :], lhsT=self.ones_col[:], rhs=gacc[:],
                             start=True, stop=True)
            rg = self.work.tile([1, 512], F32, tag="rg", bufs=2)
            nc.vector.reciprocal_approx_fast(rg[:], g[0:1, :])
            rr = self.work.tile([128, 512], F32, tag="rr", bufs=2)
            nc.gpsimd.partition_broadcast(rr[:], rg[:])
        self.pending_norms.append((b, c, h, u, rr))

    def finish(self):
        while self.pending_norms:
            self.emit_norm_pending()
        for bc in self.oproj_ready:
            self.push_oproj(*bc)
        self.oproj_ready = []
        self.fill.drain_all()


def make_in_maps(x, Wq, bq, Wk, bk, Wv, bv, Wo, bo):
    xT_np = np.ascontiguousarray(
        x.reshape(T, D_MODEL).T).astype(ml_dtypes.bfloat16)
    in_maps = []
    for c in range(N_CORES):
        sl = slice(c * JL, (c + 1) * JL)
        wqkv_np = np.concatenate(
            [Wq[:, sl], Wk[:, sl], Wv[:, sl]], axis=1).astype(ml_dtypes.bfloat16)
        # [D, 768] -> [k, p, g, c] -> [p, g, k, c]: each (g) slice becomes one
        # contiguous run per partition for fast DMA
        wqkvP = np.ascontiguousarray(
            wqkv_np.reshape(KD, 128, 6, 128).transpose(1, 2, 0, 3))
        bqkv_np = np.concatenate([bq[sl], bk[sl], bv[sl]]).astype(np.float32)
        wo_np = np.ascontiguousarray(Wo[sl, :]).astype(ml_dtypes.bfloat16)
        in_maps.append({
            "xT": xT_np, "wqkv": wqkvP, "bqkv": bqkv_np, "wo": wo_np,
        })
    return in_maps


def kernel(x, Wq, bq, Wk, bk, Wv, bv, Wo, bo):
    global _CACHED_NC
    x, Wq, bq, Wk, bk, Wv, bv, Wo, bo = [
        np.asarray(a, np.float32) for a in (x, Wq, bq, Wk, bk, Wv, bv, Wo, bo)
    ]
    if _CACHED_NC is None:
        _CACHED_NC = build_program()
    nc = _CACHED_NC

    in_maps = make_in_maps(x, Wq, bq, Wk, bk, Wv, bv, Wo, bo)
    res = run_bass_kernel_spmd(nc, in_maps, core_ids=list(range(N_CORES)))

    acc = res.results[0]["outT"].astype(np.float32)
    for c in range(1, N_CORES):
        acc += res.results[c]["outT"].astype(np.float32)
    out = acc.T + bo[None, :]
    return np.ascontiguousarray(out.reshape(B, S, D_MODEL), dtype=np.float32)


# ---------------------------------------------------------------- dev tools

def _np_partial_reference(inputs, core):
    """fp32 numpy partial output for one core's heads (no bo)."""
    x = np.asarray(inputs["x"], np.float32).reshape(T, D_MODEL)
    sl = slice(core * JL, (core + 1) * JL)
    q = x @ np.asarray(inputs["Wq"])[:, sl] + np.asarray(inputs["bq"])[sl]
    k = x @ np.asarray(inputs["Wk"])[:, sl] + np.asarray(inputs["bk"])[sl]
    v = x @ np.asarray(inputs["Wv"])[:, sl] + np.asarray(inputs["bv"])[sl]
    y = np.zeros((T, JL), np.float32)
    for b in range(B):
        tb = slice(b * S, (b + 1) * S)
        for h in range(H_PER):
            js = slice(h * HEAD_DIM, (h + 1) * HEAD_DIM)
            qh, kh, vh = q[tb, js], k[tb, js], v[tb, js]
            s = (qh @ kh.T) * SCALE
            mask = np.triu(np.ones((S, S), bool), k=1)
            s[mask] = -np.inf
            s -= s.max(axis=1, keepdims=True)
            p = np.exp(s)
            p /= p.sum(axis=1, keepdims=True)
            y[tb, js] = p @ vh
    return (y @ np.asarray(inputs["Wo"])[sl, :]).T  # [D, T]


def _simulate_core0():
    import reference
    from concourse.bass_interp import CoreSim

    inputs = {k: np.asarray(v) for k, v in reference.setup_inputs().items()}
    nc = build_program()
    in_map = make_in_maps(**inputs)[0]

    sim = CoreSim(nc)
    for name, arr in in_map.items():
        sim.tensor(name)[:] = arr
    sim.simulate(check_with_hw=False)
    got = np.asarray(sim.tensor("outT"), np.float32)

    want = _np_partial_reference(inputs, 0)
    denom = np.abs(want).max()
    err = np.abs(got - want).max() / denom
    print(f"sim core0 partial: max={np.abs(got).max():.4f} "
          f"absmax_err={np.abs(got - want).max():.5f} rel={err:.5f}")


if __name__ == "__main__":
    import sys
    if "--sim" in sys.argv:
        _simulate_core0()
    else:
        nc = build_program()
        n_inst = sum(len(bb.instructions) for bb in nc.m.functions[0].blocks)
        print(f"built: {n_inst} instructions")
